# revision 14
# baseline (speedup 1.0000x reference)
"""LocationSensitiveAttention Trainium2 Bass kernel.

Data-parallel over batch B=128 across 8 NeuronCores (16 rows/core).
All weights replicated. Full inputs in, full outputs out.

Per-core device pipeline for each batch row b (A-major energies layout,
a=attention dim on partitions):
  E[a,t] = (Wcomb^T @ X)[a,t] + pm^T[a,t]          (PE, PSUM accumulate)
  s2     = tanh(E + pq[a])                          (ACT, per-partition bias)
  e[t]   = sum_a v[a]*s2[a,t]                       (PE, 8 small matmuls -> t-major)
  u      = exp(e + maskbias), S = sum(u)            (ACT w/ fused accum)
  w      = u / S                                    (ones-matmul bcast + DVE)
  ctx    = w @ mem[b]                               (PE, PSUM accumulate)
"""

import sys

sys.path.insert(0, "/opt/trn_rl_repo")

import numpy as np

import concourse.bass as bass
import concourse.bacc as bacc
import concourse.tile as tile
import concourse.mybir as mybir
from concourse import bass_utils

N_CORES = 8
B, T, RNN, EMB, A = 128, 1024, 1024, 512, 128
NF, KS, PAD = 32, 31, 15
BPC = B // N_CORES          # batch rows per core
NCH = T // 128              # 8 t-chunks
TP = T + 2 * PAD            # 1054 padded time size

F32 = mybir.dt.float32
BF16 = mybir.dt.bfloat16

# dtype config: "f32" or "bf16" for the three big streamed tensors
import os
CFG_BF16 = os.environ.get("KERNEL_BF16", "0") == "1"


def _np_dt(dt):
    return mybir.dt.np(dt)


def _raw_ap(ap, offset_elems, dims):
    """Build a raw access pattern on ap's tensor. dims = [[step, count], ...] in elements."""
    return bass.AP(tensor=ap.tensor, offset=ap.offset + offset_elems, ap=list(dims))


def build_module(big_dt, repeat=1):
    nc = bacc.Bacc("TRN2", target_bir_lowering=False, debug=False)

    h_t = nc.dram_tensor("h_t", (BPC, RNN), F32, kind="ExternalInput").ap()
    mem_t = nc.dram_tensor("mem_t", (BPC, T, EMB), big_dt, kind="ExternalInput").ap()
    pmt_t = nc.dram_tensor("pmt_t", (BPC, A, T), big_dt, kind="ExternalInput").ap()
    awp_t = nc.dram_tensor("awp_t", (BPC, 2, TP), big_dt, kind="ExternalInput").ap()
    mask_t = nc.dram_tensor("mask_t", (BPC, T), F32, kind="ExternalInput").ap()
    wq_t = nc.dram_tensor("wq_t", (RNN, A), F32, kind="ExternalInput").ap()
    wc_t = nc.dram_tensor("wc_t", (NF, 2 * KS), F32, kind="ExternalInput").ap()
    wl_t = nc.dram_tensor("wl_t", (NF, A), F32, kind="ExternalInput").ap()
    v_t = nc.dram_tensor("v_t", (A, 1), F32, kind="ExternalInput").ap()
    id_t = nc.dram_tensor("id_t", (128, 128), F32, kind="ExternalInput").ap()
    ctx_o = nc.dram_tensor("ctx_o", (BPC, EMB), F32, kind="ExternalOutput").ap()
    w_o = nc.dram_tensor("w_o", (BPC, T), F32, kind="ExternalOutput").ap()

    with tile.TileContext(nc) as tc:
        for _ in range(repeat):
            _body(tc, h_t, mem_t, pmt_t, awp_t, mask_t, wq_t, wc_t, wl_t, v_t,
                  id_t, ctx_o, w_o, big_dt)

    nc.compile()
    return nc


def _body(tc, h_t, mem_t, pmt_t, awp_t, mask_t, wq_t, wc_t, wl_t, v_t, id_t,
          ctx_o, w_o, big_dt):
    nc = tc.nc
    from contextlib import ExitStack

    with ExitStack() as ctx:
        consts = ctx.enter_context(tc.tile_pool(name="consts", bufs=1))

        ident = consts.tile([128, 128], F32)
        nc.sync.dma_start(out=ident, in_=id_t)
        if big_dt == F32:
            ident_big = ident
        else:
            ident_big = consts.tile([128, 128], big_dt)
            nc.vector.tensor_copy(ident_big, ident)
        v_sb = consts.tile([A, 1], F32)
        nc.sync.dma_start(out=v_sb, in_=v_t)
        wc_sb = consts.tile([NF, 2 * KS], F32)
        nc.sync.dma_start(out=wc_sb, in_=wc_t)
        wl_sb = consts.tile([NF, A], F32)
        nc.sync.dma_start(out=wl_sb, in_=wl_t)

        # wq_sb[p, kc*128 + a] = Wq[kc*128 + p, a]
        wq_sb = consts.tile([128, RNN], F32)
        nc.sync.dma_start(
            out=wq_sb.rearrange("p (kc a) -> p kc a", kc=NCH),
            in_=_raw_ap(wq_t, 0, [[A, 128], [128 * A, NCH], [1, A]]),
        )
        # h loaded straight, transposed on PE below into hT_sb
        h_sb = consts.tile([BPC, RNN], F32)
        nc.sync.dma_start(out=h_sb, in_=h_t)
        hT_sb = consts.tile([128, NCH * BPC], F32)

        ones_sb = consts.tile([128, 128], F32)
        nc.vector.memset(ones_sb, 1.0)

        W2 = consts.tile([2 * KS, 128], big_dt)
        pqT = consts.tile([128, BPC], F32)
        u_all = consts.tile([128, BPC * NCH], F32)
        w_all = consts.tile([128, BPC * NCH], F32)

        with tc.tile_pool(name="ppro", bufs=1, space="PSUM") as ppro:
            wcomb_ps = ppro.tile([2 * KS, 128], F32)
            nc.tensor.matmul(wcomb_ps, lhsT=wc_sb, rhs=wl_sb, start=True, stop=True)
            nc.vector.tensor_copy(W2, wcomb_ps)

            # transpose h chunks: hT_sb[:, kc*BPC:(kc+1)*BPC] = h[:, kc*128:..].T
            for kc in range(NCH):
                hT_ps = ppro.tile([128, BPC], F32, tag="hT_ps")
                nc.tensor.transpose(hT_ps, h_sb[:, kc * 128:(kc + 1) * 128],
                                    ident[0:BPC, 0:BPC])
                nc.vector.tensor_copy(hT_sb[:, kc * BPC:(kc + 1) * BPC], hT_ps)

            pqT_ps = ppro.tile([128, BPC], F32)
            for kc in range(NCH):
                nc.tensor.matmul(
                    pqT_ps,
                    lhsT=wq_sb[:, kc * 128:(kc + 1) * 128],
                    rhs=hT_sb[:, kc * BPC:(kc + 1) * BPC],
                    start=(kc == 0),
                    stop=(kc == NCH - 1),
                )
            nc.vector.tensor_copy(pqT, pqT_ps)

        with ExitStack() as lctx:
            lx = lctx.enter_context(tc.tile_pool(name="lx", bufs=2))
            lpm = lctx.enter_context(tc.tile_pool(name="lpm", bufs=2))
            lmem = lctx.enter_context(tc.tile_pool(name="lmem", bufs=2))
            ls2 = lctx.enter_context(tc.tile_pool(name="ls2", bufs=2))
            lsmall = lctx.enter_context(tc.tile_pool(name="lsmall", bufs=2))
            pse = lctx.enter_context(tc.tile_pool(name="pse", bufs=2, space="PSUM"))
            pset = lctx.enter_context(tc.tile_pool(name="pset", bufs=1, space="PSUM"))
            psc = lctx.enter_context(tc.tile_pool(name="psc", bufs=2, space="PSUM"))

            for b in range(BPC):
                ub = u_all[:, b * NCH:(b + 1) * NCH]
                wb = w_all[:, b * NCH:(b + 1) * NCH]

                # X[(c*31+k), t] = awp[b, c, t + k]
                X = lx.tile([2 * KS, T], big_dt)
                nc.sync.dma_start(
                    out=X,
                    in_=_raw_ap(awp_t, b * 2 * TP, [[TP, 2], [1, KS], [1, T]]),
                )
                pm_sb = lpm.tile([A, T], big_dt)
                nc.sync.dma_start(
                    out=pm_sb,
                    in_=_raw_ap(pmt_t, b * A * T, [[T, A], [1, T]]),
                )
                # mem_sb[p, c, d] = mem[b, c*128 + p, d]
                mem_sb = lmem.tile([128, NCH, EMB], big_dt)
                nc.sync.dma_start(
                    out=mem_sb,
                    in_=_raw_ap(mem_t, b * T * EMB,
                                [[EMB, 128], [128 * EMB, NCH], [1, EMB]]),
                )
                # msk[p, c] = mask[b, c*128 + p]
                msk = lsmall.tile([128, NCH], F32, tag="msk")
                nc.sync.dma_start(
                    out=msk,
                    in_=_raw_ap(mask_t, b * T, [[1, 128], [128, NCH]]),
                )

                # energies accumulate: E = W2^T @ X + I^T @ pmT
                e_ps = pse.tile([128, T], F32)
                for lo in (0, 512):
                    nc.tensor.matmul(e_ps[:, lo:lo + 512], lhsT=W2,
                                     rhs=X[:, lo:lo + 512], start=True, stop=False)
                    nc.tensor.matmul(e_ps[:, lo:lo + 512], lhsT=ident_big,
                                     rhs=pm_sb[:, lo:lo + 512], start=False,
                                     stop=True)

                # s2 = tanh(E + pq_b)
                s2 = ls2.tile([128, T], F32)
                nc.scalar.activation(out=s2, in_=e_ps,
                                     func=mybir.ActivationFunctionType.Tanh,
                                     bias=pqT[:, b:b + 1], scale=1.0)

                # e[t] = sum_a v[a] s2[a, t], t-major columns
                et_ps = pset.tile([128, NCH], F32, tag="et")
                for c in range(NCH):
                    nc.tensor.matmul(et_ps[:, c:c + 1],
                                     lhsT=s2[:, c * 128:(c + 1) * 128],
                                     rhs=v_sb, start=True, stop=True)

                # mask bias and exp
                mb_f = lsmall.tile([128, NCH], F32, tag="mb")
                nc.vector.tensor_scalar_mul(out=mb_f, in0=msk, scalar1=-1e30)
                e_tm = lsmall.tile([128, NCH], F32, tag="etm")
                nc.vector.tensor_add(out=e_tm, in0=et_ps, in1=mb_f)
                ured = lsmall.tile([128, 1], F32, tag="ured")
                nc.scalar.activation(out=ub, in_=e_tm,
                                     func=mybir.ActivationFunctionType.Exp,
                                     accum_out=ured)

                # S broadcast to all partitions, then w = u / S
                sbc_ps = pset.tile([128, 1], F32, tag="sbc")
                nc.tensor.matmul(sbc_ps, lhsT=ones_sb, rhs=ured, start=True,
                                 stop=True)
                inv_c = lsmall.tile([128, 1], F32, tag="inv")
                nc.vector.reciprocal(inv_c, sbc_ps)
                nc.vector.tensor_scalar_mul(out=wb, in0=ub, scalar1=inv_c)

                # context: ctx = sum_c w_chunk^T @ mem_chunk
                if big_dt == F32:
                    w_mm = wb
                else:
                    w_mm = lsmall.tile([128, NCH], big_dt, tag="wbf")
                    nc.vector.tensor_copy(w_mm, wb)
                ctx_ps = psc.tile([1, EMB], F32)
                for c in range(NCH):
                    nc.tensor.matmul(ctx_ps, lhsT=w_mm[:, c:c + 1],
                                     rhs=mem_sb[:, c, :],
                                     start=(c == 0), stop=(c == NCH - 1))
                ctx_row = lsmall.tile([1, EMB], F32, tag="ctxr")
                nc.vector.tensor_copy(ctx_row, ctx_ps)
                nc.sync.dma_start(out=ctx_o[b:b + 1, :], in_=ctx_row)

        # transpose w_all and write out: w_o flat[i*128 + p] = w_all[p, i]
        with tc.tile_pool(name="pepi", bufs=1, space="PSUM") as pepi, \
                tc.tile_pool(name="sepi", bufs=1) as sepi:
            wT_ps = pepi.tile([128, 128], F32)
            nc.tensor.transpose(wT_ps, w_all, ident)
            wT_sb = sepi.tile([128, 128], F32)
            nc.vector.tensor_copy(wT_sb, wT_ps)
            nc.sync.dma_start(
                out=_raw_ap(w_o, 0, [[128, 128], [1, 128]]),
                in_=wT_sb,
            )


_module_cache = {}


def get_module(repeat=1):
    key = ("bf16" if CFG_BF16 else "f32", repeat)
    if key not in _module_cache:
        _module_cache[key] = build_module(BF16 if CFG_BF16 else F32, repeat)
    return _module_cache[key]


def prepare_in_maps(attention_hidden_state, memory, processed_memory,
                    attention_weights_cat, mask, W_query, W_conv, W_loc_proj, v):
    big_np = _np_dt(BF16) if CFG_BF16 else np.float32

    h = np.asarray(attention_hidden_state, np.float32)
    mem = np.asarray(memory, np.float32)
    pm = np.asarray(processed_memory, np.float32)
    aw = np.asarray(attention_weights_cat, np.float32)
    mask = np.asarray(mask)
    Wq = np.asarray(W_query, np.float32)
    Wc = np.asarray(W_conv, np.float32)
    Wl = np.asarray(W_loc_proj, np.float32)
    vv = np.asarray(v, np.float32)

    # host-side layout prep (no arithmetic)
    awp = np.zeros((B, 2, TP), np.float32)
    awp[:, :, PAD:PAD + T] = aw
    awp = awp.astype(big_np)
    pmt = np.ascontiguousarray(pm.transpose(0, 2, 1)).astype(big_np)
    mem_c = np.ascontiguousarray(mem).astype(big_np)
    mask_f = mask.astype(np.float32)
    wc_r = np.ascontiguousarray(Wc.reshape(NF, 2 * KS))
    v_c = np.ascontiguousarray(vv.reshape(A, 1))
    ident = np.eye(128, dtype=np.float32)

    in_maps = []
    for i in range(N_CORES):
        s = slice(i * BPC, (i + 1) * BPC)
        in_maps.append({
            "h_t": np.ascontiguousarray(h[s]),
            "mem_t": np.ascontiguousarray(mem_c[s]),
            "pmt_t": np.ascontiguousarray(pmt[s]),
            "awp_t": np.ascontiguousarray(awp[s]),
            "mask_t": np.ascontiguousarray(mask_f[s]),
            "wq_t": Wq,
            "wc_t": wc_r,
            "wl_t": Wl,
            "v_t": v_c,
            "id_t": ident,
        })
    return in_maps


def kernel(attention_hidden_state, memory, processed_memory,
           attention_weights_cat, mask, W_query, W_conv, W_loc_proj, v,
           trace=False, repeat=1):
    nc = get_module(repeat)
    in_maps = prepare_in_maps(
        attention_hidden_state, memory, processed_memory, attention_weights_cat,
        mask, W_query, W_conv, W_loc_proj, v)

    res = bass_utils.run_bass_kernel_spmd(
        nc, in_maps, core_ids=list(range(N_CORES)), trace=trace)

    ctx = np.concatenate([r["ctx_o"] for r in res.results], axis=0)
    w = np.concatenate([r["w_o"] for r in res.results], axis=0)
    kernel.last_results = res
    return ctx.astype(np.float32), w.astype(np.float32)
